# revision 4
# baseline (speedup 1.0000x reference)
"""Trainium2 Bass kernel for nn_DifferentiableProjector (volume rendering).

Math (per ray i, samples s=0..S-1, channels c):
    g[s]      = exp(-rho[s] * DT)
    alpha[s]  = 1 - g[s]
    T_incl[s] = prod_{s'<=s} g[s']            (+1e-10 term is negligible)
    T_excl[s] = prod_{s'<s}  g[s']
    w[s]      = T_excl[s] * alpha[s] = T_excl[s] - T_incl[s]
    out[i,c]  = sum_s w[s] * f[i,s,c]

Sharding: data-parallel over rays, 65536 rays -> 8 cores x 8192 rays.
Each core streams its f shard (64 MiB) once; per 128-ray tile:
  ACT: exp, DVE: cumprod scan + shifted sub + broadcast-mul + segment reduce.
"""

import numpy as np

import concourse.bass as bass
import concourse.tile as tile
from concourse.bacc import Bacc
from concourse import mybir
from concourse.bass_utils import run_bass_kernel_spmd

H, W, S, C = 256, 256, 128, 16
N = H * W
NCORES = 8
NS = N // NCORES          # rays per core
P = 128                   # partitions (rays per tile)
TILES = NS // P
DT = (6.0 - 2.0) / S

_cached = {}

# test-harness hooks (ignored by grading path)
TRACE = False
LAST_RESULTS = None


def _build_nc() -> bass.Bass:
    nc = Bacc()
    rho_d = nc.dram_tensor("rho", [NS, S], mybir.dt.float32, kind="ExternalInput")
    f_d = nc.dram_tensor("f", [NS, S * C], mybir.dt.float32, kind="ExternalInput")
    out_d = nc.dram_tensor("out", [NS, C], mybir.dt.float32, kind="ExternalOutput")

    with tile.TileContext(nc) as tc:
        with (
            tc.tile_pool(name="fpool", bufs=3) as fpool,
            tc.tile_pool(name="tmppool", bufs=2) as tmppool,
            tc.tile_pool(name="small", bufs=4) as small,
            tc.tile_pool(name="opool", bufs=4) as opool,
        ):
            for t in range(TILES):
                r0 = t * P
                f_t = fpool.tile([P, S * C], mybir.dt.float32)
                nc.sync.dma_start(out=f_t, in_=f_d[r0 : r0 + P, :])
                rho_t = small.tile([P, S], mybir.dt.float32)
                nc.sync.dma_start(out=rho_t, in_=rho_d[r0 : r0 + P, :])

                # g = exp(-DT * rho)
                g_t = small.tile([P, S], mybir.dt.float32)
                nc.scalar.activation(
                    out=g_t,
                    in_=rho_t,
                    func=mybir.ActivationFunctionType.Exp,
                    scale=-DT,
                )

                # Tex = [1, cumprod(g)]  (length S+1)
                tex = small.tile([P, S + 1], mybir.dt.float32)
                nc.vector.memset(tex[:, 0:1], 1.0)
                nc.vector.tensor_tensor_scan(
                    out=tex[:, 1 : S + 1],
                    data0=g_t,
                    data1=g_t,
                    initial=1.0,
                    op0=mybir.AluOpType.mult,
                    op1=mybir.AluOpType.bypass,
                )

                # w[s] = Tex[s] - Tex[s+1]  (= T_excl - T_incl)
                w_t = small.tile([P, S], mybir.dt.float32)
                nc.vector.tensor_sub(w_t, tex[:, 0:S], tex[:, 1 : S + 1])

                # tmp[i, s, c] = f[i, s, c] * w[i, s]
                tmp = tmppool.tile([P, S, C], mybir.dt.float32)
                f_v = f_t.rearrange("p (s c) -> p s c", c=C)
                nc.vector.tensor_mul(
                    tmp, f_v, w_t[:, :, None].broadcast_to((P, S, C))
                )

                # out[i, c] = sum_s tmp[i, s, c]
                o_t = opool.tile([P, C], mybir.dt.float32)
                nc.vector.tensor_reduce(
                    out=o_t,
                    in_=tmp.transpose([0, 2, 1]),
                    axis=mybir.AxisListType.X,
                    op=mybir.AluOpType.add,
                )
                nc.sync.dma_start(out=out_d[r0 : r0 + P, :], in_=o_t)
    if not nc.is_finalized():
        nc.finalize()
    return nc


def kernel(rho: np.ndarray, f: np.ndarray) -> np.ndarray:
    global LAST_RESULTS
    if "nc" not in _cached:
        _cached["nc"] = _build_nc()
    nc = _cached["nc"]

    rho2 = np.ascontiguousarray(np.asarray(rho, dtype=np.float32).reshape(N, S))
    f2 = np.ascontiguousarray(np.asarray(f, dtype=np.float32).reshape(N, S * C))
    in_maps = [
        {"rho": rho2[i * NS : (i + 1) * NS], "f": f2[i * NS : (i + 1) * NS]}
        for i in range(NCORES)
    ]
    res = run_bass_kernel_spmd(nc, in_maps, list(range(NCORES)), trace=TRACE)
    LAST_RESULTS = res
    out = np.concatenate([res.results[i]["out"] for i in range(NCORES)], axis=0)
    return (
        out.reshape(H, W, C).transpose(2, 0, 1)[None].astype(np.float32, copy=False)
    )


# revision 14
# speedup vs baseline: 1.5912x; 1.5912x over previous
"""Trainium2 Bass kernel for nn_DifferentiableProjector (volume rendering).

Math (per ray i, samples s=0..S-1, channels c):
    T_excl[s] = exp(-DT * sum_{s'<s} rho[s'])
    T_incl[s] = exp(-DT * sum_{s'<=s} rho[s'])
    w[s]      = T_excl[s] - T_incl[s]        (= T_excl * alpha)
    out[i,c]  = sum_s w[s] * f[i,s,c]

Sharding: data-parallel over rays, 65536 rays -> 8 cores x 8192 rays.

v2 design (all compute in "transposed space", s on partitions):
  - host casts f/rho to fp16; f pre-transposed to [N, C, S] so a single
    xbar-transpose DMA per 128-ray tile yields fT [s, (c, i)] in SBUF
  - cumsum over s via triangular-ones matmuls on TensorE (fp32 PSUM)
  - w_T = exp(excl) - exp(incl) on ScalarE/VectorE, [s, i] fp16
  - the big multiply runs on VectorE at 2x (fp16, broadcast over the
    OUTER free dim keeps innermost step 1)
  - segment-reduce over s = ones-vector matmul on TensorE (partition
    contraction), accumulated rows evacuated by ScalarE
  - output is produced c-major [C, rays]: exactly the final layout
"""

import numpy as np

import concourse.bass as bass
import concourse.tile as tile
from concourse.bacc import Bacc
from concourse import mybir
from concourse.bass_utils import run_bass_kernel_spmd

H, W, S, C = 256, 256, 128, 16
N = H * W
NCORES = 8
NS = N // NCORES          # rays per core
P = 128                   # partitions (rays per tile)
TILES = NS // P           # 64
GROUP = 4                 # tiles per PSUM output-row group
DT = (6.0 - 2.0) / S

_cached = {}

# test-harness hooks (ignored by grading path)
TRACE = False
LAST_RESULTS = None

F16 = mybir.dt.float16
F32 = mybir.dt.float32


def _build_nc(ns: int = NS) -> bass.Bass:
    ntiles = ns // P
    nc = Bacc()
    rho_d = nc.dram_tensor("rho", [ns, S], F16, kind="ExternalInput")
    f_d = nc.dram_tensor("f", [ns, C * S], F16, kind="ExternalInput")
    cst_d = nc.dram_tensor("consts", [P, 2 * P + C * C], F16, kind="ExternalInput")
    out_d = nc.dram_tensor("out", [C, ns], F32, kind="ExternalOutput")

    with tile.TileContext(nc) as tc:
        with (
            tc.tile_pool(name="cpool", bufs=1) as cpool,
            tc.tile_pool(name="fpool", bufs=3) as fpool,
            tc.tile_pool(name="tpool", bufs=2) as tpool,
            tc.tile_pool(name="spool", bufs=3) as spool,
            tc.tile_pool(name="opool", bufs=1) as opool,
            tc.tile_pool(name="psc", bufs=2, space="PSUM") as psc,
            tc.tile_pool(name="pso", bufs=2, space="PSUM") as pso,
        ):
            # NOTE: all xbar-transpose DMAs must issue from ONE HWDGE queue —
            # concurrent transposes from sync+scalar queues corrupt each other
            # (shared xbar). Plain copies go on the other queue.
            consts = cpool.tile([P, 2 * P + C * C], F16)
            nc.scalar.dma_start(out=consts, in_=cst_d[:, :])
            u_excl = consts[:, 0:P]
            u_incl = consts[:, P : 2 * P]
            # E_c = consts[:, 2P + 16c : 2P + 16c + 16]: column m one-hot at c
            e_base = 2 * P

            # persistent per-core output accumulator [C, ns] fp32
            out_acc = opool.tile([C, ns], F32)

            for t in range(ntiles):
                r0 = t * P

                # fT[s, c, i] <- xbar-transpose of f[r0:r0+128] ([i, (c,s)])
                fT = fpool.tile([P, C, P], F16)
                nc.sync.dma_start_transpose(out=fT, in_=f_d[r0 : r0 + P, :])
                # rhoT[s, i]
                rhoT = spool.tile([P, P], F16)
                nc.sync.dma_start_transpose(out=rhoT, in_=rho_d[r0 : r0 + P, :])

                # cumsum over s (partition axis) via triangular matmuls
                pexc = psc.tile([P, P], F32)
                pinc = psc.tile([P, P], F32)
                nc.tensor.matmul(pexc, u_excl, rhoT, start=True, stop=True)
                nc.tensor.matmul(pinc, u_incl, rhoT, start=True, stop=True)

                # exps in fp32 (w = e1 - e2 cancels; fp16 here costs ~4% on w)
                e1 = spool.tile([P, P], F32)
                e2 = spool.tile([P, P], F32)
                nc.scalar.activation(
                    e1, pexc, mybir.ActivationFunctionType.Exp, scale=-DT
                )
                nc.scalar.activation(
                    e2, pinc, mybir.ActivationFunctionType.Exp, scale=-DT
                )
                w = spool.tile([P, P], F16)
                nc.vector.tensor_sub(w, e1, e2)

                # tmp[s, c, i] = fT[s, c, i] * w[s, i]
                tmp = tpool.tile([P, C, P], F16)
                nc.vector.tensor_mul(
                    tmp, fT, w[:, None, :].broadcast_to((P, C, P))
                )

                # psum_oc[c, i] = sum_s tmp[s, c, i]: 16 accumulating one-hot
                # matmuls (lhsT column block E_c routes the sum into row c)
                psum_oc = pso.tile([C, P], F32)
                for c in range(C):
                    nc.tensor.matmul(
                        psum_oc,
                        consts[:, e_base + c * C : e_base + (c + 1) * C],
                        tmp[:, c, :],
                        start=(c == 0),
                        stop=(c == C - 1),
                    )
                # evacuate tile result into the output accumulator
                nc.scalar.activation(
                    out_acc[:, r0 : r0 + P],
                    psum_oc,
                    mybir.ActivationFunctionType.Copy,
                )
            nc.scalar.dma_start(out=out_d[:, :], in_=out_acc)
    if not nc.is_finalized():
        nc.finalize()
    return nc


def _consts() -> np.ndarray:
    u_excl = np.triu(np.ones((P, P), np.float16), 1)
    u_incl = np.triu(np.ones((P, P), np.float16), 0)
    # E[:, c*C + m] = 1 if m == c else 0  (all rows identical)
    e = np.tile(np.eye(C, dtype=np.float16).reshape(1, C * C), (P, 1))
    return np.ascontiguousarray(np.concatenate([u_excl, u_incl, e], axis=1))


def kernel(rho: np.ndarray, f: np.ndarray) -> np.ndarray:
    global LAST_RESULTS
    if "nc" not in _cached:
        _cached["nc"] = _build_nc()
        _cached["consts"] = _consts()
    nc = _cached["nc"]

    rho16 = np.asarray(rho, dtype=np.float16).reshape(N, S)
    # [N, S, C] -> [N, C, S] fp16, contiguous
    f16 = np.ascontiguousarray(
        np.asarray(f, dtype=np.float16).reshape(N, S, C).transpose(0, 2, 1)
    ).reshape(N, C * S)
    cst = _cached["consts"]

    in_maps = [
        {
            "rho": rho16[i * NS : (i + 1) * NS],
            "f": f16[i * NS : (i + 1) * NS],
            "consts": cst,
        }
        for i in range(NCORES)
    ]
    res = run_bass_kernel_spmd(nc, in_maps, list(range(NCORES)), trace=TRACE)
    LAST_RESULTS = res
    out = np.concatenate(
        [res.results[i]["out"] for i in range(NCORES)], axis=1
    )  # [C, N]
    return out.reshape(C, H, W)[None].astype(np.float32, copy=False)


# revision 15
# speedup vs baseline: 2.1734x; 1.3658x over previous
"""Trainium2 Bass kernel for nn_DifferentiableProjector (volume rendering).

Math (per ray i, samples s=0..S-1, channels c):
    T_excl[s] = exp(-DT * sum_{s'<s} rho[s'])
    T_incl[s] = exp(-DT * sum_{s'<=s} rho[s'])
    w[s]      = T_excl[s] - T_incl[s]        (= T_excl * alpha)
    out[i,c]  = sum_s w[s] * f[i,s,c]

Sharding: data-parallel over rays, 65536 rays -> 8 cores x 8192 rays.

Design (all compute in "transposed space", s on partitions):
  - host casts f/rho to fp16; f pre-transposed to [N, C, S] so a single
    xbar-transpose DMA per 512-ray tile yields fT [s, (c, i)] in SBUF
  - cumsum over s (partition axis) via triangular-ones matmuls on TensorE
    (fp32 PSUM); w = exp(excl) - exp(incl) with fp32 exps (fp16 would
    cancel), cast to fp16
  - the big multiply on VectorE at 2x (fp16; broadcast over the OUTER
    free dim keeps innermost step 1)
  - segment-reduce over s: 16 accumulating one-hot matmuls on TensorE
    route channel-c column sums into PSUM row c -> [16, T] per tile
  - ALL xbar-transpose DMAs on the sync queue (concurrent transposes
    from two HWDGE queues corrupt each other); plain copies on scalar
  - output accumulates c-major [C, 8192] in SBUF; one contiguous DMA
"""

import numpy as np

import concourse.bass as bass
import concourse.tile as tile
from concourse.bacc import Bacc
from concourse import mybir
from concourse.bass_utils import run_bass_kernel_spmd

H, W, S, C = 256, 256, 128, 16
N = H * W
NCORES = 8
NS = N // NCORES          # rays per core
P = 128                   # partitions (= S)
T = 512                   # rays per tile
DT = (6.0 - 2.0) / S

_cached = {}

# test-harness hooks (ignored by grading path)
TRACE = False
LAST_RESULTS = None

F16 = mybir.dt.float16
F32 = mybir.dt.float32


def _build_nc(ns: int = NS) -> bass.Bass:
    ntiles = ns // T
    nc = Bacc()
    rho_d = nc.dram_tensor("rho", [ns, S], F16, kind="ExternalInput")
    f_d = nc.dram_tensor("f", [ns, C * S], F16, kind="ExternalInput")
    cst_d = nc.dram_tensor("consts", [P, 2 * P + C * C], F16, kind="ExternalInput")
    out_d = nc.dram_tensor("out", [C, ns], F32, kind="ExternalOutput")

    with tile.TileContext(nc) as tc:
        with (
            tc.tile_pool(name="cpool", bufs=1) as cpool,
            tc.tile_pool(name="fpool", bufs=3) as fpool,
            tc.tile_pool(name="tpool", bufs=2) as tpool,
            tc.tile_pool(name="spool", bufs=3) as spool,
            tc.tile_pool(name="opool", bufs=1) as opool,
            tc.tile_pool(name="psc", bufs=2, space="PSUM") as psc,
            tc.tile_pool(name="pso", bufs=2, space="PSUM") as pso,
        ):
            consts = cpool.tile([P, 2 * P + C * C], F16)
            nc.scalar.dma_start(out=consts, in_=cst_d[:, :])
            u_excl = consts[:, 0:P]
            u_incl = consts[:, P : 2 * P]
            # E_c = consts[:, 2P + 16c : 2P + 16c + 16]: column m one-hot at c
            e_base = 2 * P

            # persistent per-core output accumulator [C, ns] fp32
            out_acc = opool.tile([C, ns], F32)

            for t in range(ntiles):
                r0 = t * T

                # fT[s, c, i] <- xbar-transpose of f[r0:r0+T] ([i, (c,s)])
                fT = fpool.tile([P, C, T], F16)
                nc.sync.dma_start_transpose(out=fT, in_=f_d[r0 : r0 + T, :])
                # rhoT[s, i]
                rhoT = spool.tile([P, T], F16)
                nc.sync.dma_start_transpose(out=rhoT, in_=rho_d[r0 : r0 + T, :])

                # cumsum over s (partition axis) via triangular matmuls
                pexc = psc.tile([P, T], F32)
                pinc = psc.tile([P, T], F32)
                nc.tensor.matmul(pexc, u_excl, rhoT, start=True, stop=True)
                nc.tensor.matmul(pinc, u_incl, rhoT, start=True, stop=True)

                # exps in fp32 (w = e1 - e2 cancels; fp16 here costs ~4% on w)
                e1 = spool.tile([P, T], F32)
                e2 = spool.tile([P, T], F32)
                nc.scalar.activation(
                    e1, pexc, mybir.ActivationFunctionType.Exp, scale=-DT
                )
                nc.scalar.activation(
                    e2, pinc, mybir.ActivationFunctionType.Exp, scale=-DT
                )
                w = spool.tile([P, T], F16)
                nc.vector.tensor_sub(w, e1, e2)

                # tmp[s, c, i] = fT[s, c, i] * w[s, i]
                tmp = tpool.tile([P, C, T], F16)
                nc.vector.tensor_mul(
                    tmp, fT, w[:, None, :].broadcast_to((P, C, T))
                )

                # psum_oc[c, i] = sum_s tmp[s, c, i]: 16 accumulating one-hot
                # matmuls (lhsT column block E_c routes the sum into row c)
                psum_oc = pso.tile([C, T], F32)
                for c in range(C):
                    nc.tensor.matmul(
                        psum_oc,
                        consts[:, e_base + c * C : e_base + (c + 1) * C],
                        tmp[:, c, :],
                        start=(c == 0),
                        stop=(c == C - 1),
                    )
                # evacuate tile result into the output accumulator
                nc.scalar.activation(
                    out_acc[:, r0 : r0 + T],
                    psum_oc,
                    mybir.ActivationFunctionType.Copy,
                )
            nc.scalar.dma_start(out=out_d[:, :], in_=out_acc)
    if not nc.is_finalized():
        nc.finalize()
    return nc


def _consts() -> np.ndarray:
    u_excl = np.triu(np.ones((P, P), np.float16), 1)
    u_incl = np.triu(np.ones((P, P), np.float16), 0)
    # E[:, c*C + m] = 1 if m == c else 0  (all rows identical)
    e = np.tile(np.eye(C, dtype=np.float16).reshape(1, C * C), (P, 1))
    return np.ascontiguousarray(np.concatenate([u_excl, u_incl, e], axis=1))


def kernel(rho: np.ndarray, f: np.ndarray) -> np.ndarray:
    global LAST_RESULTS
    if "nc" not in _cached:
        _cached["nc"] = _build_nc()
        _cached["consts"] = _consts()
    nc = _cached["nc"]

    rho16 = np.asarray(rho, dtype=np.float16).reshape(N, S)
    # [N, S, C] -> [N, C, S] fp16, contiguous
    f16 = np.ascontiguousarray(
        np.asarray(f, dtype=np.float16).reshape(N, S, C).transpose(0, 2, 1)
    ).reshape(N, C * S)
    cst = _cached["consts"]

    in_maps = [
        {
            "rho": rho16[i * NS : (i + 1) * NS],
            "f": f16[i * NS : (i + 1) * NS],
            "consts": cst,
        }
        for i in range(NCORES)
    ]
    res = run_bass_kernel_spmd(nc, in_maps, list(range(NCORES)), trace=TRACE)
    LAST_RESULTS = res
    out = np.concatenate(
        [res.results[i]["out"] for i in range(NCORES)], axis=1
    )  # [C, N]
    return out.reshape(C, H, W)[None].astype(np.float32, copy=False)


# revision 18
# speedup vs baseline: 2.9663x; 1.3649x over previous
"""Trainium2 Bass kernel for nn_DifferentiableProjector (volume rendering).

Math (per ray i, samples s=0..S-1, channels c):
    T_excl[s] = exp(-DT * sum_{s'<s} rho[s'])
    T_incl[s] = exp(-DT * sum_{s'<=s} rho[s'])
    w[s]      = T_excl[s] - T_incl[s]        (= T_excl * alpha)
    out[i,c]  = sum_s w[s] * f[i,s,c]

Sharding: data-parallel over rays, 65536 rays -> 8 cores x 8192 rays.

Design (all compute in "transposed space", s on partitions):
  - host casts f/rho to fp16; f pre-transposed to [N, C, S] so a single
    xbar-transpose DMA per 512-ray tile yields fT [s, (c, i)] in SBUF
  - cumsum over s (partition axis) via triangular-ones matmuls on TensorE
    (fp32 PSUM); w = exp(excl) - exp(incl) with fp32 exps (fp16 would
    cancel), cast to fp16
  - the big multiply on VectorE at 2x (fp16; broadcast over the OUTER
    free dim keeps innermost step 1)
  - segment-reduce over s: 16 accumulating one-hot matmuls on TensorE
    route channel-c column sums into PSUM row c -> [16, T] per tile
  - ALL xbar-transpose DMAs on the sync queue (concurrent transposes
    from two HWDGE queues corrupt each other); plain copies on scalar
  - output accumulates c-major [C, 8192] in SBUF; one contiguous DMA
"""

import numpy as np

import concourse.bass as bass
import concourse.tile as tile
from concourse.bacc import Bacc
from concourse import mybir
from concourse.bass_utils import run_bass_kernel_spmd

H, W, S, C = 256, 256, 128, 16
N = H * W
NCORES = 8
NS = N // NCORES          # rays per core
P = 128                   # partitions (= S)
T = 512                   # rays per tile
DT = (6.0 - 2.0) / S

_cached = {}

# test-harness hooks (ignored by grading path)
TRACE = False
LAST_RESULTS = None

F16 = mybir.dt.float16
F32 = mybir.dt.float32


def _build_nc(ns: int = NS) -> bass.Bass:
    ntiles = ns // T
    nc = Bacc()
    # host supplies both tensors pre-transposed: rho [S, rays], f [S, C, rays]
    rho_d = nc.dram_tensor("rho", [S, ns], F16, kind="ExternalInput")
    f_d = nc.dram_tensor("f", [S, C * ns], F16, kind="ExternalInput")
    cst_d = nc.dram_tensor("consts", [P, 2 * P + C * C], F16, kind="ExternalInput")
    out_d = nc.dram_tensor("out", [C, ns], F32, kind="ExternalOutput")
    f_v = f_d[:, :].rearrange("s (c i) -> s c i", i=ns)

    with tile.TileContext(nc) as tc:
        with (
            tc.tile_pool(name="cpool", bufs=1) as cpool,
            tc.tile_pool(name="fpool", bufs=3) as fpool,
            tc.tile_pool(name="tpool", bufs=2) as tpool,
            tc.tile_pool(name="spool", bufs=3) as spool,
            tc.tile_pool(name="opool", bufs=1) as opool,
            tc.tile_pool(name="psc", bufs=2, space="PSUM") as psc,
            tc.tile_pool(name="pso", bufs=2, space="PSUM") as pso,
        ):
            consts = cpool.tile([P, 2 * P + C * C], F16)
            nc.scalar.dma_start(out=consts, in_=cst_d[:, :])
            u_excl = consts[:, 0:P]
            u_incl = consts[:, P : 2 * P]
            # E_c = consts[:, 2P + 16c : 2P + 16c + 16]: column m one-hot at c
            e_base = 2 * P

            # persistent per-core output accumulator [C, ns] fp32
            out_acc = opool.tile([C, ns], F32)

            for t in range(ntiles):
                r0 = t * T

                # fT[s, c, i]: plain strided DMA (1 KB runs per (s, c));
                # alternate queues to keep both DGE streams busy
                fT = fpool.tile([P, C, T], F16)
                f_eng = nc.sync if t % 2 == 0 else nc.scalar
                f_eng.dma_start(out=fT, in_=f_v[:, :, r0 : r0 + T])
                # rhoT[s, i]
                rhoT = spool.tile([P, T], F16)
                nc.sync.dma_start(out=rhoT, in_=rho_d[:, r0 : r0 + T])

                # cumsum over s (partition axis) via triangular matmuls
                pexc = psc.tile([P, T], F32)
                pinc = psc.tile([P, T], F32)
                nc.tensor.matmul(pexc, u_excl, rhoT, start=True, stop=True)
                nc.tensor.matmul(pinc, u_incl, rhoT, start=True, stop=True)

                # exps in fp32 (w = e1 - e2 cancels; fp16 here costs ~4% on w)
                e1 = spool.tile([P, T], F32)
                e2 = spool.tile([P, T], F32)
                nc.scalar.activation(
                    e1, pexc, mybir.ActivationFunctionType.Exp, scale=-DT
                )
                nc.scalar.activation(
                    e2, pinc, mybir.ActivationFunctionType.Exp, scale=-DT
                )
                w = spool.tile([P, T], F16)
                nc.vector.tensor_sub(w, e1, e2)

                # tmp[s, c, i] = fT[s, c, i] * w[s, i]
                tmp = tpool.tile([P, C, T], F16)
                nc.vector.tensor_mul(
                    tmp, fT, w[:, None, :].broadcast_to((P, C, T))
                )

                # psum_oc[c, i] = sum_s tmp[s, c, i]: 16 accumulating one-hot
                # matmuls (lhsT column block E_c routes the sum into row c)
                psum_oc = pso.tile([C, T], F32)
                for c in range(C):
                    nc.tensor.matmul(
                        psum_oc,
                        consts[:, e_base + c * C : e_base + (c + 1) * C],
                        tmp[:, c, :],
                        start=(c == 0),
                        stop=(c == C - 1),
                    )
                # evacuate tile result into the output accumulator
                nc.scalar.activation(
                    out_acc[:, r0 : r0 + T],
                    psum_oc,
                    mybir.ActivationFunctionType.Copy,
                )
            nc.scalar.dma_start(out=out_d[:, :], in_=out_acc)
    if not nc.is_finalized():
        nc.finalize()
    return nc


def _consts() -> np.ndarray:
    u_excl = np.triu(np.ones((P, P), np.float16), 1)
    u_incl = np.triu(np.ones((P, P), np.float16), 0)
    # E[:, c*C + m] = 1 if m == c else 0  (all rows identical)
    e = np.tile(np.eye(C, dtype=np.float16).reshape(1, C * C), (P, 1))
    return np.ascontiguousarray(np.concatenate([u_excl, u_incl, e], axis=1))


def kernel(rho: np.ndarray, f: np.ndarray) -> np.ndarray:
    global LAST_RESULTS
    if "nc" not in _cached:
        _cached["nc"] = _build_nc()
        _cached["consts"] = _consts()
    nc = _cached["nc"]

    rho16 = np.asarray(rho, dtype=np.float16).reshape(N, S)
    f16 = np.asarray(f, dtype=np.float16).reshape(N, S, C)
    cst = _cached["consts"]

    in_maps = []
    for i in range(NCORES):
        sl = slice(i * NS, (i + 1) * NS)
        # [rays, S] -> [S, rays]; [rays, S, C] -> [S, C, rays]
        rho_t = np.ascontiguousarray(rho16[sl].T)
        f_t = np.ascontiguousarray(f16[sl].transpose(1, 2, 0)).reshape(S, C * NS)
        in_maps.append({"rho": rho_t, "f": f_t, "consts": cst})
    res = run_bass_kernel_spmd(nc, in_maps, list(range(NCORES)), trace=TRACE)
    LAST_RESULTS = res
    out = np.concatenate(
        [res.results[i]["out"] for i in range(NCORES)], axis=1
    )  # [C, N]
    return out.reshape(C, H, W)[None].astype(np.float32, copy=False)
